# revision 5
# baseline (speedup 1.0000x reference)
"""Trainium2 Bass kernel for the entmax-bisect Tsallis loss (nn_BisectionLoss).

Math: for each row, the reference runs a 50-step f32 bisection on
f(t) = sum(relu(Xs - t)^(1/(V-1))) - 1 with Xs = 0.5*X.  Because the exponent
1/(V-1) = 1/31999 is tiny, every element strictly above t contributes a value
in [0.9968, 1) and every other element contributes exactly 0, so f(t) >= 0
exactly when at least TWO elements exceed t.  The bisection decision at every
step is therefore [x2 > t] where x2 is the row's second-largest element, and
the final distribution p is supported only on elements within one f32 ulp
below x2 (top-2 in practice, top-8 with huge margin).

Device work per core (memory-bound, one pass over X):
  1. Stream X in [128, 16000] chunks; DVE Max8 gives each row's top-8 values
     (multiset, descending -- ties preserved).
  2. Exact f32 bisection on [128, NT] per-row scalars using the x2 decision,
     mirroring the reference's f32 arithmetic op-for-op.
  3. Sparse loss evaluation on the top-8 values:
     Z = relu(Xs - t)^eps (via ACT ln/exp), p = Z/sum(Z),
     loss = (1 - sum(p^1.5))/0.75 + dot(p, X_top8) - X[row, target].
Sharding: rows split evenly across the 8 cores; no communication.
"""

from contextlib import ExitStack

import numpy as np

B, V = 4096, 32000
NCORES = 8
RB = B // NCORES  # 512 rows per core
P = 128
NT = RB // P  # 4 row-tiles per core
CHUNK = 16000  # Max8 input free-size limit is 16384
NCH = V // CHUNK  # 2 chunks per row
N_ITER = 50
ALPHA = 1.5
EPS = np.float32(1.0 / (V - 1))
CVAL = np.float32(V ** (1.0 - ALPHA))
INV_DENOM = np.float32(1.0 / (ALPHA * (ALPHA - 1.0)))  # 1/0.75

_CACHE: dict = {}


def _build():
    import concourse.bass as bass  # noqa: F401
    import concourse.tile as tile
    from concourse import bacc, mybir

    f32 = mybir.dt.float32
    AX = mybir.AxisListType.X
    Alu = mybir.AluOpType
    Act = mybir.ActivationFunctionType

    nc = bacc.Bacc(
        "TRN2", target_bir_lowering=False, debug=False, enable_asserts=False
    )
    Xp = nc.declare_dram_parameter("X", [RB, V], f32, isOutput=False)
    XTp = nc.declare_dram_parameter("XT", [RB], f32, isOutput=False)
    OUTp = nc.declare_dram_parameter("OUT", [RB], f32, isOutput=True)
    X = Xp.ap()

    with tile.TileContext(nc) as tc, ExitStack() as ctx:
        xpool = ctx.enter_context(tc.tile_pool(name="xc", bufs=2))
        sp = ctx.enter_context(tc.tile_pool(name="small", bufs=1))
        lp = ctx.enter_context(tc.tile_pool(name="loss", bufs=2))

        cand = sp.tile([P, NT * NCH * 8], f32)
        top8 = sp.tile([P, NT * 8], f32)
        for j in range(NT):
            for c in range(NCH):
                xt_ = xpool.tile([P, CHUNK], f32)
                nc.sync.dma_start(
                    xt_[:], X[j * P : (j + 1) * P, c * CHUNK : (c + 1) * CHUNK]
                )
                k = (j * NCH + c) * 8
                nc.vector.max(cand[:, k : k + 8], xt_[:])
            nc.vector.max(
                top8[:, j * 8 : (j + 1) * 8],
                cand[:, j * NCH * 8 : (j + 1) * NCH * 8],
            )

        # Xs = (alpha-1)*top8 = 0.5*top8 (exact).  m/x2 are strided views.
        Xs = sp.tile([P, NT * 8], f32)
        nc.vector.tensor_scalar_mul(Xs[:], top8[:], 0.5)
        m = Xs[:][:, 0 : NT * 8 : 8]  # [P, NT] row maxima
        x2 = Xs[:][:, 1 : NT * 8 : 8]  # [P, NT] second-largest

        tmin = sp.tile([P, NT], f32)
        tmax = sp.tile([P, NT], f32)
        diff0 = sp.tile([P, NT], f32)
        t = sp.tile([P, NT], f32)
        mask = sp.tile([P, NT], mybir.dt.uint8)
        nc.vector.tensor_scalar_sub(tmin[:], m, 1.0)
        nc.vector.tensor_scalar_sub(tmax[:], m, float(CVAL))
        nc.vector.tensor_sub(diff0[:], tmax[:], tmin[:])
        for i in range(N_ITER):
            # t = tmin + diff0*2^-(i+1); the scale is exact so this matches
            # the reference's running diff-halving bit-for-bit.
            nc.vector.scalar_tensor_tensor(
                out=t[:],
                in0=diff0[:],
                scalar=float(2.0 ** -(i + 1)),
                in1=tmin[:],
                op0=Alu.mult,
                op1=Alu.add,
            )
            nc.vector.tensor_tensor(mask[:], x2, t[:], Alu.is_gt)
            nc.vector.select(tmin[:], mask[:], t[:], tmin[:])
        # t holds the final iteration's threshold (what the reference's last
        # body evaluation used for Z).

        xt = sp.tile([P, NT], f32)
        nc.sync.dma_start(xt[:], XTp.ap().rearrange("(j p) -> p j", p=P))
        lossT = sp.tile([P, NT], f32)

        for j in range(NT):
            v8 = top8[:, j * 8 : (j + 1) * 8]
            xs8 = Xs[:, j * 8 : (j + 1) * 8]
            tj = t[:, j : j + 1]
            u = lp.tile([P, 8], f32)
            nc.vector.tensor_scalar(
                out=u[:], in0=xs8, scalar1=tj, scalar2=0.0,
                op0=Alu.subtract, op1=Alu.max,
            )
            msk = lp.tile([P, 8], f32)
            nc.vector.tensor_scalar(
                out=msk[:], in0=u[:], scalar1=0.0, scalar2=None, op0=Alu.is_gt
            )
            # Clamp before ln so u=0 lanes stay finite; they are zeroed by msk.
            uc = lp.tile([P, 8], f32)
            nc.vector.tensor_scalar_max(uc[:], u[:], 1e-38)
            lnu = lp.tile([P, 8], f32)
            nc.scalar.activation(lnu[:], uc[:], Act.Ln)
            Zr = lp.tile([P, 8], f32)
            nc.scalar.activation(Zr[:], lnu[:], Act.Exp, scale=float(EPS))
            Z = lp.tile([P, 8], f32)
            nc.vector.tensor_mul(Z[:], Zr[:], msk[:])
            S1 = lp.tile([P, 1], f32)
            nc.vector.reduce_sum(S1[:], Z[:], axis=AX)
            rcp = lp.tile([P, 1], f32)
            nc.vector.reciprocal(rcp[:], S1[:])
            p = lp.tile([P, 8], f32)
            nc.vector.tensor_scalar_mul(p[:], Z[:], rcp[:])
            sq = lp.tile([P, 8], f32)
            nc.scalar.activation(sq[:], p[:], Act.Sqrt)
            pa = lp.tile([P, 8], f32)
            nc.vector.tensor_mul(pa[:], p[:], sq[:])
            Sa = lp.tile([P, 1], f32)
            nc.vector.reduce_sum(Sa[:], pa[:], axis=AX)
            q = lp.tile([P, 1], f32)
            nc.vector.tensor_scalar(
                out=q[:], in0=Sa[:], scalar1=1.0, scalar2=float(INV_DENOM),
                op0=Alu.subtract, op1=Alu.mult,
            )
            Dd = lp.tile([P, 1], f32)
            prod = lp.tile([P, 8], f32)
            nc.vector.tensor_mul(prod[:], p[:], v8)
            nc.vector.reduce_sum(Dd[:], prod[:], axis=AX)
            tmp = lp.tile([P, 1], f32)
            nc.vector.tensor_sub(tmp[:], Dd[:], q[:])
            nc.vector.tensor_sub(lossT[:, j : j + 1], tmp[:], xt[:, j : j + 1])

        nc.sync.dma_start(OUTp.ap().rearrange("(j p) -> p j", p=P), lossT[:])

    nc.compile()
    return nc


def get_nc():
    if "nc" not in _CACHE:
        _CACHE["nc"] = _build()
    return _CACHE["nc"]


def kernel(X: np.ndarray, target: np.ndarray) -> np.ndarray:
    from concourse.bass_utils import run_bass_kernel_spmd

    X = np.ascontiguousarray(np.asarray(X, dtype=np.float32))
    target = np.asarray(target)
    assert X.shape == (B, V) and target.shape == (B,)

    xt = X[np.arange(B), target.astype(np.int64)].astype(np.float32)

    nc = get_nc()
    in_maps = [
        {
            "X": X[c * RB : (c + 1) * RB],
            "XT": xt[c * RB : (c + 1) * RB],
        }
        for c in range(NCORES)
    ]
    res = run_bass_kernel_spmd(nc, in_maps, core_ids=list(range(NCORES))).results
    return np.concatenate([res[c]["OUT"] for c in range(NCORES)], axis=0)
